# revision 1
# baseline (speedup 1.0000x reference)
"""Trainium2 Bass kernel for nn_HabitatGraph (gnn_message_passing).

Full-input contract: kernel(**inputs) takes the complete arrays, shards the
batch (graph) dimension B=256 across 8 NeuronCores (32 graphs each), runs one
SPMD NEFF via run_bass_kernel_spmd, and gathers the full [256,256,256] output.

Math (reference.py, exploiting that dist_mat is symmetric and >= 0 by
construction, so the to_undirected mean reduces to dist itself):
  sim  = cosine_similarity(x_g)                    # [H,H] per graph
  pm   = mask_i * mask_j * (1 - eye)               # undirected edge mask
  out  = pm * relu(sim) * exp(-dist^2 / (sigma^2 + EPS))
sigma is a GLOBAL (whole-batch) std over masked dist entries -> three scalar
sums; computed on host and passed in as one broadcast constant.
"""

import numpy as np
import ml_dtypes
from contextlib import ExitStack

from concourse import bacc, bass, mybir, tile
from concourse.bass_utils import run_bass_kernel_spmd

N_CORES = 8
B, H, FEAT = 256, 256, 512
SHARD = B // N_CORES          # 32 graphs per core
KC = FEAT // 128              # 4 k-chunks of the contraction dim
EPS = 1e-6

F32 = mybir.dt.float32
BF16 = mybir.dt.bfloat16
AF = mybir.ActivationFunctionType


def build_nc():
    nc = bacc.Bacc("TRN2", debug=False, num_devices=N_CORES)

    xt = nc.dram_tensor("xt", [SHARD, FEAT, H], F32, kind="ExternalInput").ap()
    dist = nc.dram_tensor("dist", [SHARD, H, H], F32, kind="ExternalInput").ap()
    mrow = nc.dram_tensor("mrow", [SHARD, H], BF16, kind="ExternalInput").ap()
    eyec = nc.dram_tensor("eyec", [H, H], F32, kind="ExternalInput").ap()
    scal = nc.dram_tensor("scal", [128, 1], F32, kind="ExternalInput").ap()
    out = nc.dram_tensor("out", [SHARD, H, H], F32, kind="ExternalOutput").ap()

    # DRAM-side layouts: partition-major views for clean [128, ...] DMAs
    xt_r = xt.rearrange("g (c p) h -> g p c h", p=128)      # [32,128,4,256]
    dist_r = dist.rearrange("g (r p) h -> g p r h", p=128)  # [32,128,2,256]
    out_r = out.rearrange("g (r p) h -> g p r h", p=128)    # [32,128,2,256]
    eyec_r = eyec.rearrange("(r p) h -> p r h", p=128)      # [128,2,256]

    with tile.TileContext(nc) as tc, ExitStack() as ctx:
        const = ctx.enter_context(tc.tile_pool(name="const", bufs=1))
        xpool = ctx.enter_context(tc.tile_pool(name="x", bufs=3))
        xqpool = ctx.enter_context(tc.tile_pool(name="xq", bufs=2))
        xnpool = ctx.enter_context(tc.tile_pool(name="xn", bufs=2))
        dpool = ctx.enter_context(tc.tile_pool(name="d", bufs=3))
        mpool = ctx.enter_context(tc.tile_pool(name="m", bufs=3))
        spool = ctx.enter_context(tc.tile_pool(name="s", bufs=3))
        epool = ctx.enter_context(tc.tile_pool(name="e", bufs=4))
        opool = ctx.enter_context(tc.tile_pool(name="o", bufs=4))
        ps_n = ctx.enter_context(tc.tile_pool(name="psn", bufs=2, space="PSUM"))
        ps_s = ctx.enter_context(tc.tile_pool(name="pss", bufs=2, space="PSUM"))
        ps_p = ctx.enter_context(tc.tile_pool(name="psp", bufs=2, space="PSUM"))

        eyec_t = const.tile([128, 2, H], F32)
        nc.sync.dma_start(eyec_t[:], eyec_r[:])
        scal_t = const.tile([128, 1], F32)
        nc.sync.dma_start(scal_t[:], scal[:])
        ones_t = const.tile([128, 1], BF16)
        nc.vector.memset(ones_t[:], 1.0)

        for g in range(SHARD):
            # ---- load x^T for this graph: [128 (f), 4 (k-chunk), 256 (h)]
            xtile = xpool.tile([128, KC, H], F32, tag="xtile")
            nc.sync.dma_start(xtile[:], xt_r[g])

            # ---- squared entries (bf16 is plenty for norms)
            xsq = xqpool.tile([128, KC, H], BF16, tag="xsq")
            nc.scalar.activation(xsq[:], xtile[:], AF.Square)

            # ---- column norms via ones-matmul: nrm[1,h] = sum_f x[f,h]^2
            nrm = ps_n.tile([1, H], F32, tag="nrm")
            for c in range(KC):
                nc.tensor.matmul(nrm[:], ones_t[:], xsq[:, c, :],
                                 start=(c == 0), stop=(c == KC - 1))

            # ---- s[h] = 1/sqrt(max(nrm,1e-24)); then broadcast to 128 parts
            smax = spool.tile([1, H], F32, tag="smax")
            nc.vector.tensor_scalar_max(smax[:], nrm[:], 1e-24)
            srec = spool.tile([1, H], F32, tag="srec")
            nc.vector.reciprocal(srec[:], smax[:])
            srow = spool.tile([1, H], F32, tag="srow")
            nc.scalar.activation(srow[:], srec[:], AF.Sqrt)
            sful = spool.tile([128, H], F32, tag="sful")
            nc.gpsimd.partition_broadcast(sful[:], srow[:])

            # ---- normalized x^T in bf16
            xn = xnpool.tile([128, KC, H], BF16, tag="xn")
            for c in range(KC):
                nc.vector.tensor_mul(xn[:, c, :], xtile[:, c, :], sful[:])

            # ---- mask row for this graph
            mr = mpool.tile([1, H], BF16, tag="mr")
            nc.sync.dma_start(mr[:], mrow[g : g + 1, :])

            # ---- distances [128, 2 (row-half), 256]
            dtile = dpool.tile([128, 2, H], F32, tag="dtile")
            nc.sync.dma_start(dtile[:], dist_r[g])
            sqd = dpool.tile([128, 2, H], F32, tag="sqd")
            nc.scalar.activation(sqd[:], dtile[:], AF.Square)

            for h in range(2):
                # sim half: [128, 256] = xn[:, :, h*128:+128].T @ xn
                sim = ps_s.tile([128, H], F32, tag="sim")
                for c in range(KC):
                    nc.tensor.matmul(
                        sim[:],
                        xn[:, c, h * 128 : (h + 1) * 128],
                        xn[:, c, :],
                        start=(c == 0),
                        stop=(c == KC - 1),
                    )
                # pair-mask half via K=1 outer product
                pm = ps_p.tile([128, H], F32, tag="pm")
                nc.tensor.matmul(pm[:], mr[:, h * 128 : (h + 1) * 128], mr[:])

                pmz = epool.tile([128, H], BF16, tag="pmz")
                nc.vector.tensor_mul(pmz[:], pm[:], eyec_t[:, h, :])

                ew = epool.tile([128, H], BF16, tag="ew")
                nc.scalar.activation(ew[:], sqd[:, h, :], AF.Exp, scale=scal_t[:])

                rl = epool.tile([128, H], BF16, tag="rl")
                nc.scalar.activation(rl[:], sim[:], AF.Relu)

                se = epool.tile([128, H], BF16, tag="se")
                nc.vector.tensor_mul(se[:], rl[:], ew[:])

                ot = opool.tile([128, H], F32, tag="ot")
                nc.vector.tensor_mul(ot[:], se[:], pmz[:])
                nc.sync.dma_start(out_r[g][:, h, :], ot[:])

    nc.compile()
    return nc


_NC = None


def _get_nc():
    global _NC
    if _NC is None:
        _NC = build_nc()
    return _NC


def make_in_maps(x_feat, dist_mat, mask):
    x = np.asarray(x_feat, np.float32).reshape(B, H, FEAT)
    dist = np.ascontiguousarray(np.asarray(dist_mat, np.float32))
    mf = np.asarray(mask).astype(np.float32)

    # global sigma: unbiased std over masked undirected edge weights.
    # pm[b,i,j] = mask_i*mask_j*(1-eye); dist symmetric >= 0 by construction.
    mf64 = mf.astype(np.float64)
    d64 = dist.astype(np.float64)
    k = mf64.sum(1)
    n = float((k * k - k).sum())
    t1 = np.einsum("bij,bj->bi", d64, mf64)
    s1_full = float((t1 * mf64).sum())
    diag = np.einsum("bii->bi", d64)
    s1 = s1_full - float((diag * mf64).sum())
    d2 = d64 * d64
    t2 = np.einsum("bij,bj->bi", d2, mf64)
    s2_full = float((t2 * mf64).sum())
    diag2 = np.einsum("bii->bi", d2)
    s2 = s2_full - float((diag2 * mf64).sum())
    mean = s1 / max(n, 1.0)
    var = (s2 - n * mean * mean) / max(n - 1.0, 1.0)
    sigma = max(np.sqrt(max(var, 0.0)), EPS)
    neg_inv = np.float32(-1.0 / (sigma * sigma + EPS))

    eyec = (1.0 - np.eye(H, dtype=np.float32))
    scal = np.full((128, 1), neg_inv, np.float32)
    mrow_bf = mf.astype(ml_dtypes.bfloat16)

    in_maps = []
    for c in range(N_CORES):
        sl = slice(c * SHARD, (c + 1) * SHARD)
        xt = np.ascontiguousarray(x[sl].transpose(0, 2, 1))  # [32, 512, 256]
        in_maps.append(
            {
                "xt": xt,
                "dist": dist[sl],
                "mrow": np.ascontiguousarray(mrow_bf[sl]),
                "eyec": eyec,
                "scal": scal,
            }
        )
    return in_maps


def kernel(x_feat, dist_mat, mask):
    nc = _get_nc()
    in_maps = make_in_maps(x_feat, dist_mat, mask)
    res = run_bass_kernel_spmd(nc, in_maps, core_ids=list(range(N_CORES)))
    return np.concatenate([res.results[c]["out"] for c in range(N_CORES)], axis=0)


# revision 4
# speedup vs baseline: 1.1108x; 1.1108x over previous
"""Trainium2 Bass kernel for nn_HabitatGraph (gnn_message_passing).

Full-input contract: kernel(**inputs) takes the complete arrays, shards the
batch (graph) dimension B=256 across 8 NeuronCores (32 graphs each), runs one
SPMD NEFF via run_bass_kernel_spmd, and gathers the full [256,256,256] output.

Math (reference.py, exploiting that dist_mat is symmetric and >= 0 by
construction, so the to_undirected mean reduces to dist itself):
  sim  = cosine_similarity(x_g)                    # [H,H] per graph
  pm   = mask_i * mask_j * (1 - eye)               # undirected edge mask
  out  = pm * relu(sim) * exp(-dist^2 / (sigma^2 + EPS))
sigma is a GLOBAL (whole-batch) std over masked dist entries -> three scalar
sums; computed on host and passed in as one broadcast constant.
"""

import numpy as np
import ml_dtypes
from contextlib import ExitStack

from concourse import bacc, bass, mybir, tile
from concourse.bass_utils import run_bass_kernel_spmd

N_CORES = 8
B, H, FEAT = 256, 256, 512
SHARD = B // N_CORES          # 32 graphs per core
KC = FEAT // 128              # 4 k-chunks of the contraction dim
EPS = 1e-6

F32 = mybir.dt.float32
BF16 = mybir.dt.bfloat16
AF = mybir.ActivationFunctionType


def build_nc():
    nc = bacc.Bacc("TRN2", debug=False, num_devices=N_CORES)

    xt = nc.dram_tensor("xt", [SHARD, FEAT, H], F32, kind="ExternalInput").ap()
    dist = nc.dram_tensor("dist", [SHARD, H, H], F32, kind="ExternalInput").ap()
    mrow = nc.dram_tensor("mrow", [SHARD, H], BF16, kind="ExternalInput").ap()
    eyec = nc.dram_tensor("eyec", [H, H], F32, kind="ExternalInput").ap()
    scal = nc.dram_tensor("scal", [128, 1], F32, kind="ExternalInput").ap()
    out = nc.dram_tensor("out", [SHARD, H, H], F32, kind="ExternalOutput").ap()

    # DRAM-side layouts: partition-major views for clean [128, ...] DMAs
    xt_r = xt.rearrange("g (c p) h -> g p c h", p=128)      # [32,128,4,256]
    dist_r = dist.rearrange("g (r p) h -> g p r h", p=128)  # [32,128,2,256]
    out_r = out.rearrange("g (r p) h -> g p r h", p=128)    # [32,128,2,256]
    eyec_r = eyec.rearrange("(r p) h -> p r h", p=128)      # [128,2,256]

    with tile.TileContext(nc) as tc, ExitStack() as ctx:
        const = ctx.enter_context(tc.tile_pool(name="const", bufs=1))
        xpool = ctx.enter_context(tc.tile_pool(name="x", bufs=3))
        xqpool = ctx.enter_context(tc.tile_pool(name="xq", bufs=2))
        xnpool = ctx.enter_context(tc.tile_pool(name="xn", bufs=2))
        dpool = ctx.enter_context(tc.tile_pool(name="d", bufs=3))
        mpool = ctx.enter_context(tc.tile_pool(name="m", bufs=3))
        spool = ctx.enter_context(tc.tile_pool(name="s", bufs=3))
        epool = ctx.enter_context(tc.tile_pool(name="e", bufs=4))
        opool = ctx.enter_context(tc.tile_pool(name="o", bufs=4))
        ps_n = ctx.enter_context(tc.tile_pool(name="psn", bufs=2, space="PSUM"))
        ps_s = ctx.enter_context(tc.tile_pool(name="pss", bufs=2, space="PSUM"))
        ps_p = ctx.enter_context(tc.tile_pool(name="psp", bufs=2, space="PSUM"))

        eyec_t = const.tile([128, 2, H], F32)
        nc.sync.dma_start(eyec_t[:], eyec_r[:])
        scal_t = const.tile([128, 1], F32)
        nc.sync.dma_start(scal_t[:], scal[:])
        ones_t = const.tile([128, 1], BF16)
        nc.vector.memset(ones_t[:], 1.0)
        tiny_t = const.tile([1, 1], F32)
        nc.vector.memset(tiny_t[:], 1e-30)

        for g in range(SHARD):
            # ---- load x^T for this graph: [128 (f), 4 (k-chunk), 256 (h)]
            xtile = xpool.tile([128, KC, H], F32, tag="xtile")
            nc.sync.dma_start(xtile[:], xt_r[g])

            # ---- squared entries (bf16 is plenty for norms)
            xsq = xqpool.tile([128, KC, H], BF16, tag="xsq")
            nc.scalar.activation(xsq[:], xtile[:], AF.Square)

            # ---- column norms via ones-matmul: nrm[1,h] = sum_f x[f,h]^2
            nrm = ps_n.tile([1, H], F32, tag="nrm")
            for c in range(KC):
                nc.tensor.matmul(nrm[:], ones_t[:], xsq[:, c, :],
                                 start=(c == 0), stop=(c == KC - 1))

            # ---- s[h] = 1/sqrt(nrm) = exp(-0.5*ln(nrm+tiny)); Ln and Exp live
            # in the same activation table set, so no per-graph table reload.
            lnv = spool.tile([1, H], F32, tag="lnv")
            nc.scalar.activation(lnv[:], nrm[:], AF.Ln, bias=tiny_t[:])
            srow = spool.tile([1, H], F32, tag="srow")
            nc.scalar.activation(srow[:], lnv[:], AF.Exp, scale=-0.5)
            sful = spool.tile([128, H], F32, tag="sful")
            nc.gpsimd.partition_broadcast(sful[:], srow[:])

            # ---- normalized x^T in bf16
            xn = xnpool.tile([128, KC, H], BF16, tag="xn")
            for c in range(KC):
                nc.vector.tensor_mul(xn[:, c, :], xtile[:, c, :], sful[:])

            # ---- mask row for this graph
            mr = mpool.tile([1, H], BF16, tag="mr")
            nc.sync.dma_start(mr[:], mrow[g : g + 1, :])

            # ---- distances [128, 2 (row-half), 256]
            dtile = dpool.tile([128, 2, H], F32, tag="dtile")
            nc.sync.dma_start(dtile[:], dist_r[g])
            sqd = dpool.tile([128, 2, H], F32, tag="sqd")
            nc.scalar.activation(sqd[:], dtile[:], AF.Square)

            for h in range(2):
                # sim half: [128, 256] = xn[:, :, h*128:+128].T @ xn
                sim = ps_s.tile([128, H], F32, tag="sim")
                for c in range(KC):
                    nc.tensor.matmul(
                        sim[:],
                        xn[:, c, h * 128 : (h + 1) * 128],
                        xn[:, c, :],
                        start=(c == 0),
                        stop=(c == KC - 1),
                    )
                # pair-mask half via K=1 outer product
                pm = ps_p.tile([128, H], F32, tag="pm")
                nc.tensor.matmul(pm[:], mr[:, h * 128 : (h + 1) * 128], mr[:])

                pmz = epool.tile([128, H], BF16, tag="pmz")
                nc.vector.tensor_mul(pmz[:], pm[:], eyec_t[:, h, :])

                ew = epool.tile([128, H], BF16, tag="ew")
                nc.scalar.activation(ew[:], sqd[:, h, :], AF.Exp, scale=scal_t[:])

                rl = epool.tile([128, H], BF16, tag="rl")
                nc.scalar.activation(rl[:], sim[:], AF.Relu)

                se = epool.tile([128, H], BF16, tag="se")
                nc.vector.tensor_mul(se[:], rl[:], ew[:])

                ot = opool.tile([128, H], F32, tag="ot")
                nc.vector.tensor_mul(ot[:], se[:], pmz[:])
                nc.sync.dma_start(out_r[g][:, h, :], ot[:])

    nc.compile()
    return nc


_NC = None


def _get_nc():
    global _NC
    if _NC is None:
        _NC = build_nc()
    return _NC


def make_in_maps(x_feat, dist_mat, mask):
    x = np.asarray(x_feat, np.float32).reshape(B, H, FEAT)
    dist = np.ascontiguousarray(np.asarray(dist_mat, np.float32))
    mf = np.asarray(mask).astype(np.float32)

    # global sigma: unbiased std over masked undirected edge weights.
    # pm[b,i,j] = mask_i*mask_j*(1-eye); dist symmetric >= 0 by construction.
    mf64 = mf.astype(np.float64)
    d64 = dist.astype(np.float64)
    k = mf64.sum(1)
    n = float((k * k - k).sum())
    t1 = np.einsum("bij,bj->bi", d64, mf64)
    s1_full = float((t1 * mf64).sum())
    diag = np.einsum("bii->bi", d64)
    s1 = s1_full - float((diag * mf64).sum())
    d2 = d64 * d64
    t2 = np.einsum("bij,bj->bi", d2, mf64)
    s2_full = float((t2 * mf64).sum())
    diag2 = np.einsum("bii->bi", d2)
    s2 = s2_full - float((diag2 * mf64).sum())
    mean = s1 / max(n, 1.0)
    var = (s2 - n * mean * mean) / max(n - 1.0, 1.0)
    sigma = max(np.sqrt(max(var, 0.0)), EPS)
    neg_inv = np.float32(-1.0 / (sigma * sigma + EPS))

    eyec = (1.0 - np.eye(H, dtype=np.float32))
    scal = np.full((128, 1), neg_inv, np.float32)
    mrow_bf = mf.astype(ml_dtypes.bfloat16)

    in_maps = []
    for c in range(N_CORES):
        sl = slice(c * SHARD, (c + 1) * SHARD)
        xt = np.ascontiguousarray(x[sl].transpose(0, 2, 1))  # [32, 512, 256]
        in_maps.append(
            {
                "xt": xt,
                "dist": dist[sl],
                "mrow": np.ascontiguousarray(mrow_bf[sl]),
                "eyec": eyec,
                "scal": scal,
            }
        )
    return in_maps


def kernel(x_feat, dist_mat, mask):
    nc = _get_nc()
    in_maps = make_in_maps(x_feat, dist_mat, mask)
    res = run_bass_kernel_spmd(nc, in_maps, core_ids=list(range(N_CORES)))
    return np.concatenate([res.results[c]["out"] for c in range(N_CORES)], axis=0)


# revision 10
# speedup vs baseline: 2.0494x; 1.8450x over previous
"""Trainium2 Bass kernel for nn_HabitatGraph (gnn_message_passing).

Full-input contract: kernel(**inputs) takes the complete arrays, shards the
batch (graph) dimension B=256 across 8 NeuronCores (32 graphs each), runs one
SPMD NEFF via run_bass_kernel_spmd, and gathers the full [256,256,256] output.

Math (reference.py, exploiting that dist_mat is symmetric and >= 0 by
construction, so to_undirected's mean reduces to dist itself):
  sim  = cosine_similarity(x_g)                    # [H,H] per graph
  out  = m_i * m_j * (1-eye) * relu(sim) * exp(-dist^2 / (sigma^2 + EPS))
sigma is a GLOBAL (whole-batch) std over masked dist entries -> three scalar
sums; computed on host and passed in as one broadcast constant.

Device-side folds that shape the kernel:
 - 1/sqrt(v) = exp(-0.5*ln(v)): Ln+Exp live in ONE activation table set
   (natural_log_exp_and_others), so no per-op table reloads; the table list
   is patched so the compiler can only pick that set.
 - mask_j and the column norm scale are fused: s'_j = m_j / ||x_j||, applied
   to x^T before the gram matmul, so masked columns of sim are already 0.
 - mask_i + relu fused into the PSUM read: (G * m_i) max 0 via tensor_scalar.
 - eye removed by poisoning the dist diagonal on host (exp arg -> -inf -> 0).
 - bf16 end to end (inputs converted on host; output upcast on host); DRAM
   layouts are partition-major so every DMA moves contiguous >=1KB rows.
"""

import numpy as np
import ml_dtypes
from contextlib import ExitStack

import concourse.bacc as bacc_mod
from concourse import bacc, bass, mybir, tile
from concourse.bass_utils import run_bass_kernel_spmd

N_CORES = 8
B, H, FEAT = 256, 256, 512
SHARD = B // N_CORES          # 32 graphs per core
KC = FEAT // 128              # 4 k-chunks of the contraction dim
EPS = 1e-6
DIAG_POISON = 1.0e4           # exp(-poison^2/sigma^2) == 0.0

F32 = mybir.dt.float32
BF16 = mybir.dt.bfloat16
AF = mybir.ActivationFunctionType
ALU = mybir.AluOpType

_orig_get_tables = bacc_mod.get_activation_tables


def _only_nl_exp_tables(arch):
    """Keep act_func_set indices intact but blank every set except
    natural_log_exp_and_others, so insert_act_table_loads emits exactly one
    table load for our {Ln, Exp, Relu, Square} usage."""
    tabs = dict(_orig_get_tables(arch))
    return {
        name: (fns if name == "natural_log_exp_and_others" else set())
        for name, fns in tabs.items()
    }


def build_nc():
    bacc_mod.get_activation_tables = _only_nl_exp_tables
    try:
        nc = bacc.Bacc("TRN2", debug=False, num_devices=N_CORES)

        # partition-major host layouts: [g, p, c, h] so each DMA partition row
        # is one contiguous chunk.
        xt = nc.dram_tensor("xt", [SHARD, 128, KC, H], BF16, kind="ExternalInput").ap()
        dist = nc.dram_tensor("dist", [SHARD, 128, 2, H], BF16, kind="ExternalInput").ap()
        mrow = nc.dram_tensor("mrow", [SHARD, H], BF16, kind="ExternalInput").ap()
        mcolt = nc.dram_tensor("mcolt", [SHARD, 128, 2], F32, kind="ExternalInput").ap()
        scal = nc.dram_tensor("scal", [128, 1], F32, kind="ExternalInput").ap()
        out = nc.dram_tensor("out", [SHARD, 128, 2, H], BF16, kind="ExternalOutput").ap()

        mrow_r = mrow.unsqueeze(0)                     # [1, 32, 256]
        mcolt_r = mcolt.rearrange("g p h -> p g h")    # [128, 32, 2]

        with tile.TileContext(nc) as tc, ExitStack() as ctx:
            const = ctx.enter_context(tc.tile_pool(name="const", bufs=1))
            xpool = ctx.enter_context(tc.tile_pool(name="x", bufs=8))
            xqpool = ctx.enter_context(tc.tile_pool(name="xq", bufs=3))
            xnpool = ctx.enter_context(tc.tile_pool(name="xn", bufs=3))
            dpool = ctx.enter_context(tc.tile_pool(name="d", bufs=3))
            spool = ctx.enter_context(tc.tile_pool(name="s", bufs=3))
            epool = ctx.enter_context(tc.tile_pool(name="e", bufs=4))
            opool = ctx.enter_context(tc.tile_pool(name="o", bufs=4))
            ps_n = ctx.enter_context(tc.tile_pool(name="psn", bufs=2, space="PSUM"))
            ps_s = ctx.enter_context(tc.tile_pool(name="pss", bufs=3, space="PSUM"))

            scal_t = const.tile([128, 1], F32)
            nc.sync.dma_start(scal_t[:], scal[:])
            ones_t = const.tile([128, 1], BF16)
            nc.vector.memset(ones_t[:], 1.0)
            tiny_t = const.tile([1, 1], F32)
            nc.vector.memset(tiny_t[:], 1e-30)
            mrt = const.tile([1, SHARD, H], BF16)
            nc.sync.dma_start(mrt[:], mrow_r[:])
            mct = const.tile([128, SHARD, 2], F32)
            nc.sync.dma_start(mct[:], mcolt_r[:])

            for b4 in range(SHARD // 4):
                # ---- phase 1: x loads + squared column norms for 4 graphs
                xts = []
                nrm4 = ps_n.tile([1, 4, H], F32, tag="nrm4")
                for r4 in range(4):
                    g = b4 * 4 + r4
                    xtile = xpool.tile([128, KC, H], BF16, tag="xtile")
                    nc.sync.dma_start(xtile[:], xt[g])
                    xts.append(xtile)
                    xsq = xqpool.tile([128, KC, H], BF16, tag="xsq")
                    nc.vector.tensor_mul(xsq[:], xtile[:], xtile[:])
                    for c in range(KC):
                        nc.tensor.matmul(nrm4[:, r4, :], ones_t[:], xsq[:, c, :],
                                         start=(c == 0), stop=(c == KC - 1))

                # ---- batched rsqrt via Ln/Exp (one table set), mask_j folded in
                lnv = spool.tile([1, 4, H], F32, tag="lnv")
                nc.scalar.activation(lnv[:], nrm4[:], AF.Ln, bias=tiny_t[:])
                sr4 = spool.tile([1, 4, H], BF16, tag="sr4")
                nc.scalar.activation(sr4[:], lnv[:], AF.Exp, scale=-0.5)
                s4 = spool.tile([1, 4, H], BF16, tag="s4")
                nc.vector.tensor_mul(s4[:], sr4[:], mrt[:, b4 * 4 : b4 * 4 + 4, :])

                # ---- phase 2: per-graph sim + edge weights
                for r4 in range(4):
                    g = b4 * 4 + r4
                    sful = spool.tile([128, H], BF16, tag="sful")
                    nc.gpsimd.partition_broadcast(sful[:], s4[:, r4, :])

                    xn = xnpool.tile([128, KC, H], BF16, tag="xn")
                    for c in range(KC):
                        nc.vector.tensor_mul(xn[:, c, :], xts[r4][:, c, :], sful[:])

                    dtile = dpool.tile([128, 2, H], BF16, tag="dtile")
                    nc.sync.dma_start(dtile[:], dist[g])
                    sqd = dpool.tile([128, 2, H], BF16, tag="sqd")
                    nc.vector.tensor_mul(sqd[:], dtile[:], dtile[:])
                    ew = epool.tile([128, 2, H], BF16, tag="ew")
                    nc.scalar.activation(ew[:], sqd[:], AF.Exp, scale=scal_t[:])

                    sim = ps_s.tile([128, 2, H], F32, tag="sim")
                    for h in range(2):
                        for c in range(KC):
                            nc.tensor.matmul(
                                sim[:, h, :],
                                xn[:, c, h * 128 : (h + 1) * 128],
                                xn[:, c, :],
                                start=(c == 0),
                                stop=(c == KC - 1),
                            )

                    # (G * m_i) max 0  — mask_i + relu folded into the PSUM read
                    rl = epool.tile([128, 2, H], BF16, tag="rl")
                    for h in range(2):
                        nc.vector.tensor_scalar(
                            rl[:, h, :], sim[:, h, :],
                            mct[:, g, h : h + 1], 0.0,
                            op0=ALU.mult, op1=ALU.max,
                        )

                    ot = opool.tile([128, 2, H], BF16, tag="ot")
                    nc.vector.tensor_mul(ot[:], rl[:], ew[:])
                    nc.sync.dma_start(out[g], ot[:])

        nc.compile()
        return nc
    finally:
        bacc_mod.get_activation_tables = _orig_get_tables


_NC = None


def _get_nc():
    global _NC
    if _NC is None:
        _NC = build_nc()
    return _NC


def make_in_maps(x_feat, dist_mat, mask):
    x = np.asarray(x_feat, np.float32).reshape(B, H, FEAT)
    dist = np.asarray(dist_mat, np.float32)
    mf = np.asarray(mask).astype(np.float32)

    # global sigma: unbiased std over masked undirected edge weights.
    # pm[b,i,j] = mask_i*mask_j*(1-eye); dist symmetric >= 0 by construction.
    mf64 = mf.astype(np.float64)
    d64 = dist.astype(np.float64)
    k = mf64.sum(1)
    n = float((k * k - k).sum())
    t1 = np.einsum("bij,bj->bi", d64, mf64)
    s1 = float((t1 * mf64).sum()) - float((np.einsum("bii->bi", d64) * mf64).sum())
    d2 = d64 * d64
    t2 = np.einsum("bij,bj->bi", d2, mf64)
    s2 = float((t2 * mf64).sum()) - float((np.einsum("bii->bi", d2) * mf64).sum())
    mean = s1 / max(n, 1.0)
    var = (s2 - n * mean * mean) / max(n - 1.0, 1.0)
    sigma = max(np.sqrt(max(var, 0.0)), EPS)
    neg_inv = np.float32(-1.0 / (sigma * sigma + EPS))

    scal = np.full((128, 1), neg_inv, np.float32)
    mrow_bf = mf.astype(ml_dtypes.bfloat16)

    in_maps = []
    for c in range(N_CORES):
        sl = slice(c * SHARD, (c + 1) * SHARD)
        # x^T per graph, partition-major: [g, p(128), c(4), h(256)]
        xt = (
            x[sl]
            .transpose(0, 2, 1)              # [32, 512, 256]
            .reshape(SHARD, KC, 128, H)
            .transpose(0, 2, 1, 3)           # [32, 128, 4, 256]
        ).astype(ml_dtypes.bfloat16)
        db = dist[sl].copy()
        ii = np.arange(H)
        db[:, ii, ii] = DIAG_POISON          # kills self-loops via exp -> 0
        db = (
            db.reshape(SHARD, 2, 128, H).transpose(0, 2, 1, 3)  # [32,128,2,256]
        ).astype(ml_dtypes.bfloat16)
        mcolt = np.ascontiguousarray(
            mf[sl].reshape(SHARD, 2, 128).transpose(0, 2, 1)    # [32,128,2]
        )
        in_maps.append(
            {
                "xt": np.ascontiguousarray(xt),
                "dist": np.ascontiguousarray(db),
                "mrow": np.ascontiguousarray(mrow_bf[sl]),
                "mcolt": mcolt,
                "scal": scal,
            }
        )
    return in_maps


def kernel(x_feat, dist_mat, mask):
    nc = _get_nc()
    in_maps = make_in_maps(x_feat, dist_mat, mask)
    res = run_bass_kernel_spmd(nc, in_maps, core_ids=list(range(N_CORES)))
    o = np.concatenate([res.results[c]["out"] for c in range(N_CORES)], axis=0)
    # [256,128,2,256] partition-major bf16 -> [256,256,256] f32
    return o.transpose(0, 2, 1, 3).reshape(B, H, H).astype(np.float32)
